# revision 7
# baseline (speedup 1.0000x reference)
"""AUC-like pairwise loss on 8 Trainium2 NeuronCores (Bass/Tile).

Computes  cost = -mean_{i,j} sigmoid(p_i * p_j) * relu(t_i - t_j)
for N = 16384 without materializing the N x N matrices in HBM.

Math: sigmoid(p_i p_j) is symmetric in (i, j) and
relu(t_i - t_j) + relu(t_j - t_i) = |t_i - t_j|, so the full double sum
equals sum over unordered pairs {i<j} of sigmoid(p_i p_j) * |t_i - t_j|:
only half the N^2 sigmoid evaluations are needed.

Block decomposition (B = 128 blocks of 128 rows): strip I covers
column-blocks J = I..I+63 (mod B), plus J = I+64 when I < 64; the
diagonal block (J = I) gets weight 1/2 (it double-counts its pairs).
Every unordered block pair is covered exactly once.  Core c owns strips
I = c + 8k, k = 0..15.  Host-side rotation by 128*c makes the per-core
programs identical (SPMD): strip k reads columns [1024k, 1024k + W_k)
of a rotated vector of length 23552, W_k = 8320 (k<8) or 8192.

Per strip [128 rows x W cols], entirely on-chip:
  ACT: s = sigmoid(p_bcast * p_col)           one instr, per-partition scale
  DVE: custom fused op  ABS_MUL_RED_ANT:
       out = |t_bcast - t_col| * s  (discarded), accum = row-sum -> accums[:,k]
  DVE: same op on the first 128 cols (diagonal block) -> accums[:,16+k],
       which the host weights by -1/2.
Broadcasts p_bcast/t_bcast are partition-stride-0 DMA reads of the
rotated fp16 vectors.  Host sums the 8 x [128,32] accumulators in f64.
"""

import numpy as np
from contextlib import ExitStack

N = 16384
N_CORES = 8
NSTRIPS = 16
EXT = 23552            # rotated vector length
_PROGRAM = None
_OP = None


def _register_absmul_op():
    """Register the fused DVE op: out = |in0 - s0| * in1, accum_out = rowsum."""
    global _OP
    if _OP is not None:
        return _OP
    from concourse import dve_ops
    from concourse.dve_spec import Spec, Src0, Src1, C0, maxx, lower, _has_src1
    from concourse.dve_uop import DveOpSpec
    from operator import add

    name = "ABS_MUL_RED_ANT"
    for op in dve_ops.OPS:
        if op.name == name:
            _OP = op
            return op

    def _ref(in0, in1, s0, s1, imm2):
        b = (np.abs(in0.astype(np.float32) - s0) * in1).astype(np.float32)
        return b, b.reshape(b.shape[0], -1).sum(axis=-1, keepdims=True)

    spec = Spec(body=maxx(Src0 - C0, C0 - Src0) * Src1, accum=add, reference=_ref)
    opcode = max(dve_ops._SUB_OPCODE_FOR_NAME.values()) + 1
    dve_ops._SUB_OPCODE_FOR_NAME[name] = opcode
    shas = {}
    for ver in ("v3", "v4"):
        uops = lower(spec, ver=ver)
        shas[ver] = DveOpSpec(
            name=name, opcode=opcode, uops=uops, rd1_en=_has_src1(spec)
        ).sha(ver)
    op = dve_ops.DveOp(name, spec, subdim=False, uops_sha=shas)
    dve_ops.OPS.append(op)
    dve_ops.CUSTOM_DVE_SPECS[name] = spec
    _OP = op
    return op


def _build_program():
    import concourse.bass as bass
    import concourse.tile as tile
    from concourse import bacc, mybir

    op = _register_absmul_op()
    f16 = mybir.dt.float16
    f32 = mybir.dt.float32
    ACTF = mybir.ActivationFunctionType

    nc = bacc.Bacc(trn_type="TRN2", enable_asserts=False)

    p_ext = nc.dram_tensor("p_ext", [1, EXT], f16, kind="ExternalInput")
    t_ext = nc.dram_tensor("t_ext", [1, EXT], f16, kind="ExternalInput")
    # [:, k] = p[128*(c+8k)+q]; [:, 16+k] = t[...]
    cols = nc.dram_tensor("cols", [128, 2 * NSTRIPS], f32, kind="ExternalInput")
    out = nc.dram_tensor("out", [128, 2 * NSTRIPS], f32, kind="ExternalOutput")

    with ExitStack() as ctx:
        tc = ctx.enter_context(tile.TileContext(nc))
        pool = ctx.enter_context(tc.tile_pool(name="main", bufs=1))

        cols_sb = pool.tile([128, 2 * NSTRIPS], f32)
        nc.sync.dma_start(cols_sb[:], cols.ap())

        p_b = pool.tile([128, EXT], f16)
        t_b = pool.tile([128, EXT], f16)
        # broadcast DMA in pieces, earliest columns first so strip 0 can start
        pieces = [(0, 8320), (8320, 8064), (16384, 7168)]
        for (o, w) in pieces:
            nc.sync.dma_start(
                p_b[:, o:o + w], p_ext.ap()[:, o:o + w].broadcast_to((128, w))
            )
            nc.sync.dma_start(
                t_b[:, o:o + w], t_ext.ap()[:, o:o + w].broadcast_to((128, w))
            )

        accums = pool.tile([128, 2 * NSTRIPS], f32)
        junk = pool.tile([128, 8320], f16)
        spool = ctx.enter_context(tc.tile_pool(name="s", bufs=3))

        for k in range(NSTRIPS):
            W = 8320 if k < 8 else 8192
            c0 = 1024 * k
            pcol = cols_sb[:, k:k + 1]
            tcol = cols_sb[:, NSTRIPS + k:NSTRIPS + k + 1]
            s = spool.tile([128, W], f16, tag="s")
            nc.scalar.activation(s[:], p_b[:, c0:c0 + W], ACTF.Sigmoid,
                                 bias=0.0, scale=pcol)
            nc.vector._custom_dve(
                op, out=junk[:, :W], in0=t_b[:, c0:c0 + W], in1=s[:],
                s0=tcol, accum_out=accums[:, k:k + 1],
            )
            # diagonal block: host subtracts half of this partial
            nc.vector._custom_dve(
                op, out=junk[:, :128], in0=t_b[:, c0:c0 + 128], in1=s[:, 0:128],
                s0=tcol, accum_out=accums[:, NSTRIPS + k:NSTRIPS + k + 1],
            )

        nc.sync.dma_start(out.ap(), accums[:])

    nc.compile()
    return nc


def _host_inputs(y_true, y_pred):
    p = np.asarray(y_pred, dtype=np.float32).reshape(-1)
    t = np.asarray(y_true, dtype=np.float32).reshape(-1)
    assert p.shape == (N,) and t.shape == (N,)
    in_maps = []
    base = np.arange(EXT)
    for c in range(N_CORES):
        idx = (128 * c + base) % N
        cols = np.empty((128, 2 * NSTRIPS), dtype=np.float32)
        for k in range(NSTRIPS):
            i0 = 128 * (c + 8 * k)
            cols[:, k] = p[i0:i0 + 128]
            cols[:, NSTRIPS + k] = t[i0:i0 + 128]
        in_maps.append({
            "p_ext": p[idx].astype(np.float16).reshape(1, EXT),
            "t_ext": t[idx].astype(np.float16).reshape(1, EXT),
            "cols": cols,
        })
    return in_maps


def _get_program():
    global _PROGRAM
    if _PROGRAM is None:
        _PROGRAM = _build_program()
    return _PROGRAM


def run_on_cores(y_true, y_pred, trace=False):
    import concourse.bass_utils as bass_utils

    nc = _get_program()
    in_maps = _host_inputs(y_true, y_pred)
    return bass_utils.run_bass_kernel_spmd(
        nc, in_maps, core_ids=list(range(N_CORES)), trace=trace
    )


def combine(res):
    total = np.float64(0.0)
    for c in range(N_CORES):
        acc = np.asarray(res.results[c]["out"], dtype=np.float64)
        total += acc[:, :NSTRIPS].sum() - 0.5 * acc[:, NSTRIPS:].sum()
    return np.float32(-(total / (float(N) * float(N))))


def kernel(y_true, y_pred):
    return combine(run_on_cores(y_true, y_pred))


# revision 8
# speedup vs baseline: 1.0257x; 1.0257x over previous
"""AUC-like pairwise loss on 8 Trainium2 NeuronCores (Bass/Tile).

Computes  cost = -mean_{i,j} sigmoid(p_i * p_j) * relu(t_i - t_j)
for N = 16384 without materializing the N x N matrices in HBM.

Math: sigmoid(p_i p_j) is symmetric in (i, j) and
relu(t_i - t_j) + relu(t_j - t_i) = |t_i - t_j|, so the full double sum
equals the sum over unordered pairs {i<j} of sigmoid(p_i p_j)*|t_i-t_j|:
only half the N^2 sigmoid evaluations are needed.

Block decomposition (B = 128 blocks of 128 rows): strip I covers
column-blocks J = I..I+63 (mod B), plus J = I+64 when I < 64; the
diagonal block (J = I) gets weight 1/2 (it double-counts its pairs).
Every unordered block pair is covered exactly once.  Core c owns strips
I = c + 8k, k = 0..15.  Host-side rotation by 128*c makes the per-core
programs identical (SPMD): strip k reads columns [1024k, 1024k + W_k)
of a rotated fp16 vector of length 23552, W_k = 8320 (k<8) or 8192.

Per strip [128 rows x W cols], entirely on-chip (path A):
  ACT: s = sigmoid(p_bcast * p_col)          one instr, per-partition scale
  DVE: custom fused op ABS_MUL_RED_ANT:
       out = |t_bcast - t_col| * s (discarded), accum = row-sum
DVE is the bottleneck (1 elem/cycle fused op), so three strips run
path B instead, using the otherwise-idle ACT headroom and TensorE:
  ACT: s = sigmoid(...);  m = Abs(-t_bcast + t_col)   (bias AP, scale=-1)
  DVE: prod = s * m              (tensor_tensor, 2x fp16)
  PE : psum[1,512] += ones^T @ prod   (accumulating matmuls)
The diagonal-block corrections (-1/2 weight) are tiny custom-op calls
on the first 128 columns; the host applies the weights in f64.
Broadcasts are partition-stride-0 DMA reads of the rotated vectors,
staged in column pieces so compute starts after ~1 MB.
"""

import numpy as np
from contextlib import ExitStack

N = 16384
N_CORES = 8
NSTRIPS = 16
EXT = 23552
PATH_B = (9, 12, 15)          # strips computed via ACT-abs + TT + PE reduce
S0_SPLIT = 4                  # strip 0 sub-chunks (cuts startup latency)
NACC = 2 * NSTRIPS + (S0_SPLIT - 1)   # 35 accumulator slots
_PROGRAM = None
_OP = None


def _register_absmul_op():
    """Register fused DVE op: out = |in0 - s0| * in1, accum_out = rowsum."""
    global _OP
    if _OP is not None:
        return _OP
    from concourse import dve_ops
    from concourse.dve_spec import Spec, Src0, Src1, C0, maxx, lower, _has_src1
    from concourse.dve_uop import DveOpSpec
    from operator import add

    name = "ABS_MUL_RED_ANT"
    for op in dve_ops.OPS:
        if op.name == name:
            _OP = op
            return op

    def _ref(in0, in1, s0, s1, imm2):
        b = (np.abs(in0.astype(np.float32) - s0) * in1).astype(np.float32)
        return b, b.reshape(b.shape[0], -1).sum(axis=-1, keepdims=True)

    spec = Spec(body=maxx(Src0 - C0, C0 - Src0) * Src1, accum=add, reference=_ref)
    opcode = max(dve_ops._SUB_OPCODE_FOR_NAME.values()) + 1
    dve_ops._SUB_OPCODE_FOR_NAME[name] = opcode
    shas = {}
    for ver in ("v3", "v4"):
        uops = lower(spec, ver=ver)
        shas[ver] = DveOpSpec(
            name=name, opcode=opcode, uops=uops, rd1_en=_has_src1(spec)
        ).sha(ver)
    op = dve_ops.DveOp(name, spec, subdim=False, uops_sha=shas)
    dve_ops.OPS.append(op)
    dve_ops.CUSTOM_DVE_SPECS[name] = spec
    _OP = op
    return op


def _build_program():
    import concourse.bass as bass
    import concourse.tile as tile
    from concourse import bacc, mybir

    op = _register_absmul_op()
    f16 = mybir.dt.float16
    f32 = mybir.dt.float32
    A = mybir.AluOpType
    ACTF = mybir.ActivationFunctionType

    nc = bacc.Bacc(trn_type="TRN2", enable_asserts=False)

    p_ext = nc.dram_tensor("p_ext", [1, EXT], f16, kind="ExternalInput")
    t_ext = nc.dram_tensor("t_ext", [1, EXT], f16, kind="ExternalInput")
    cols = nc.dram_tensor("cols", [128, 2 * NSTRIPS], f32, kind="ExternalInput")
    out = nc.dram_tensor("out", [128, NACC], f32, kind="ExternalOutput")
    out_b = nc.dram_tensor("out_b", [1, 512], f32, kind="ExternalOutput")

    with ExitStack() as ctx:
        tc = ctx.enter_context(tile.TileContext(nc))
        pool = ctx.enter_context(tc.tile_pool(name="main", bufs=1))

        cols_sb = pool.tile([128, 2 * NSTRIPS], f32)
        nc.sync.dma_start(cols_sb[:], cols.ap())
        ones = pool.tile([128, 1], f16)
        nc.vector.memset(ones[:], 1.0)

        p_b = pool.tile([128, EXT], f16)
        t_b = pool.tile([128, EXT], f16)
        pieces = [(0, 2080), (2080, 2080), (4160, 2080), (6240, 2080),
                  (8320, 4160), (12480, 4160), (16640, 3456), (20096, 3456)]
        for (o, w) in pieces:
            nc.sync.dma_start(
                p_b[:, o:o + w], p_ext.ap()[:, o:o + w].broadcast_to((128, w)))
            nc.sync.dma_start(
                t_b[:, o:o + w], t_ext.ap()[:, o:o + w].broadcast_to((128, w)))

        accums = pool.tile([128, NACC], f32)
        junk = pool.tile([128, 8320], f16)
        spool = ctx.enter_context(tc.tile_pool(name="s", bufs=3))
        mpool = ctx.enter_context(tc.tile_pool(name="m", bufs=1))
        ppool = ctx.enter_context(tc.tile_pool(name="prod", bufs=1))
        psum = ctx.enter_context(tc.tile_pool(name="psum", bufs=1, space="PSUM"))
        acc_b = psum.tile([128, 512], f32)

        n_mm = len(PATH_B) * 16
        mm_idx = 0

        for k in range(NSTRIPS):
            W = 8320 if k < 8 else 8192
            c0 = 1024 * k
            pcol = cols_sb[:, k:k + 1]
            tcol = cols_sb[:, NSTRIPS + k:NSTRIPS + k + 1]

            if k == 0:
                # split into sub-chunks so compute starts after one DMA piece
                sub = [(0, 2080), (2080, 2080), (4160, 2080), (6240, 2080)]
                s0_first = None
                for j, (so, sw) in enumerate(sub):
                    s = spool.tile([128, sw], f16, tag="s")
                    nc.scalar.activation(s[:], p_b[:, so:so + sw], ACTF.Sigmoid,
                                         bias=0.0, scale=pcol)
                    if j == 0:
                        s0_first = s
                    slot = 0 if j == 0 else 2 * NSTRIPS + j - 1
                    nc.vector._custom_dve(
                        op, out=junk[:, :sw], in0=t_b[:, so:so + sw], in1=s[:],
                        s0=tcol, accum_out=accums[:, slot:slot + 1])
                nc.vector._custom_dve(
                    op, out=junk[:, :128], in0=t_b[:, 0:128],
                    in1=s0_first[:, 0:128], s0=tcol,
                    accum_out=accums[:, NSTRIPS:NSTRIPS + 1])
                continue

            s = spool.tile([128, W], f16, tag="s")
            nc.scalar.activation(s[:], p_b[:, c0:c0 + W], ACTF.Sigmoid,
                                 bias=0.0, scale=pcol)
            if k in PATH_B:
                m = mpool.tile([128, W], f16, tag="m")
                nc.scalar.activation(m[:], t_b[:, c0:c0 + W], ACTF.Abs,
                                     bias=tcol, scale=-1.0)
                prod = ppool.tile([128, W], f16, tag="prod")
                nc.vector.tensor_tensor(prod[:], s[:], m[:], op=A.mult)
                for b0 in range(0, W, 512):
                    nc.tensor.matmul(
                        acc_b[0:1, :], lhsT=ones[:], rhs=prod[:, b0:b0 + 512],
                        start=(mm_idx == 0), stop=(mm_idx == n_mm - 1),
                        skip_group_check=True)
                    mm_idx += 1
            else:
                nc.vector._custom_dve(
                    op, out=junk[:, :W], in0=t_b[:, c0:c0 + W], in1=s[:],
                    s0=tcol, accum_out=accums[:, k:k + 1])
            # diagonal block correction (host weights by -1/2)
            nc.vector._custom_dve(
                op, out=junk[:, :128], in0=t_b[:, c0:c0 + 128], in1=s[:, 0:128],
                s0=tcol, accum_out=accums[:, NSTRIPS + k:NSTRIPS + k + 1])
        assert mm_idx == n_mm

        res_b = pool.tile([1, 512], f32)
        nc.scalar.copy(res_b[:], acc_b[0:1, :])
        nc.sync.dma_start(out_b.ap(), res_b[:])
        nc.sync.dma_start(out.ap(), accums[:])

    nc.compile()
    return nc


def _host_inputs(y_true, y_pred):
    p = np.asarray(y_pred, dtype=np.float32).reshape(-1)
    t = np.asarray(y_true, dtype=np.float32).reshape(-1)
    assert p.shape == (N,) and t.shape == (N,)
    in_maps = []
    base = np.arange(EXT)
    for c in range(N_CORES):
        idx = (128 * c + base) % N
        cols = np.empty((128, 2 * NSTRIPS), dtype=np.float32)
        for k in range(NSTRIPS):
            i0 = 128 * (c + 8 * k)
            cols[:, k] = p[i0:i0 + 128]
            cols[:, NSTRIPS + k] = t[i0:i0 + 128]
        in_maps.append({
            "p_ext": p[idx].astype(np.float16).reshape(1, EXT),
            "t_ext": t[idx].astype(np.float16).reshape(1, EXT),
            "cols": cols,
        })
    return in_maps


def _get_program():
    global _PROGRAM
    if _PROGRAM is None:
        _PROGRAM = _build_program()
    return _PROGRAM


def run_on_cores(y_true, y_pred, trace=False, tmpdir=None):
    import concourse.bass_utils as bass_utils

    nc = _get_program()
    in_maps = _host_inputs(y_true, y_pred)
    return bass_utils.run_bass_kernel_spmd(
        nc, in_maps, core_ids=list(range(N_CORES)), trace=trace, tmpdir=tmpdir
    )


def combine(res):
    total = np.float64(0.0)
    for c in range(N_CORES):
        acc = np.asarray(res.results[c]["out"], dtype=np.float64)
        total += acc[:, :NSTRIPS].sum()              # path-A strip partials
        total += acc[:, 2 * NSTRIPS:].sum()          # strip-0 extra sub-chunks
        total -= 0.5 * acc[:, NSTRIPS:2 * NSTRIPS].sum()   # diagonal blocks
        total += np.asarray(res.results[c]["out_b"], dtype=np.float64).sum()
    return np.float32(-(total / (float(N) * float(N))))


def kernel(y_true, y_pred):
    return combine(run_on_cores(y_true, y_pred))
